# revision 19
# baseline (speedup 1.0000x reference)
"""Grouped-expert SwiGLU kernel: fp8 DoubleRow 3-term matmuls with
packed hi/lo DRAM layouts.

Every GEMM operand is decomposed into e4m3 hi+lo parts (x and weights on
the host, the gated hidden h on device). Each bf16 matmul is replaced by
three fp8 DoubleRow matmuls covering hi*hi + hi*lo + lo*hi (lo*lo is
negligible), which reconstructs bf16-level accuracy (measured rel err
~2.8e-3) while DoubleRow pairs two k-chunks per pass, cutting stage PE
time to 0.75x of the bf16 schedule. Weights are pre-scaled by 2^7 so
their hi/lo parts stay in e4m3's normal range; the scale is folded back
in the ACT/DVE ops that read PSUM.

The hi and lo parts are packed into per-partition-contiguous DRAM
tensors laid out in SBUF tile order, so every load is one DMA with
large (>=512B) contiguous runs: x is [nt*128, 32*MT] loaded with a
single DMA per slot, a w1/w3 set is [128, 64*F] loaded per k-group,
w2 is [128, 8*D] loaded in hi/lo halves. Stage 2 runs its first f-pair
for every token block before any second f-pair so the PE has cover
while the last f-chunk's gate chain (sigmoid -> mul -> mul -> fp8
split) drains.

Slot structure: tokens are tile-balanced across 8 cores, each core's
slots span at most two experts (A then B), slots below n_a_fixed are
compiled unconditionally against set A, the last slot against set B,
and only middle slots pay the If/Else cost. The B-set weight loads are
spread across several early slots so they never starve the per-slot x
loads. A warmup matmul chain burns the PE pstate ramp during the
prologue DMA wait.
"""

import math
import os

import ml_dtypes
import numpy as np

D = 2048
F = 512
MT = 256
TS = MT // 128
KC = D // 128   # 16 k-chunks of 128
KP = KC // 2    # 8 DoubleRow k-pairs
FC = F // 128   # 4 f-chunks
FP = FC // 2    # 2 DoubleRow f-pairs
NG = 4          # k-groups (4 chunks each)
NCORES = 8
WS = 128.0      # weight pre-scale (2^7)

F8 = ml_dtypes.float8_e4m3  # TRN FP8_EXP4 (max 240)

_cache = {}


def _build(nt: int, naf: int | None = None):
    import concourse.bacc as bacc
    import concourse.mybir as mybir
    from concourse.tile import TileContext

    dt = mybir.dt
    f32 = dt.float32
    bf16 = dt.bfloat16
    f8 = dt.float8e4
    i32 = dt.int32
    DR = mybir.MatmulPerfMode.DoubleRow
    ACT = mybir.ActivationFunctionType
    PAD_T = nt * MT

    nc = bacc.Bacc(
        "TRN2", target_bir_lowering=False, debug=False,
        enable_asserts=False, num_devices=NCORES,
    )

    # xp row (m*128+p) holds slot m / partition p's whole x chunk in SBUF
    # tile order: cols = (g, hl, chunk, t) -> 32*MT contiguous bytes. Large
    # contiguous runs keep the DMA model at full bus width (512B+ elements).
    xp = nc.dram_tensor("xp", [nt * 128, 32 * MT], f8, kind="ExternalInput")
    wd = {}
    for s in ("a", "b"):
        # w13p row p = cols (g, t, chunk, f), t in (1h,1l,3h,3l)
        wd[s + "13"] = nc.dram_tensor(f"w{s}13", [128, 64 * F],
                                      f8, kind="ExternalInput")
        # w2p row p = cols (hl, fchunk, d)
        wd[s + "2"] = nc.dram_tensor(f"w{s}2", [128, 8 * D],
                                     f8, kind="ExternalInput")
    meta = nc.dram_tensor("meta", [1, 1], i32, kind="ExternalInput")
    out = nc.dram_tensor("out", [PAD_T, D], bf16, kind="ExternalOutput")

    n_a_fixed = max(1, math.ceil(nt / 2)) if nt >= 2 else nt
    if naf is not None:
        n_a_fixed = max(n_a_fixed, min(naf, nt - 1))
    has_b_slot = nt >= 2

    with TileContext(nc) as tc:
        with (
            tc.tile_pool(name="wpool", bufs=1) as wpool,
            tc.tile_pool(name="xt", bufs=3) as xt_pool,
            tc.tile_pool(name="ht", bufs=4) as ht_pool,
            tc.tile_pool(name="tmp", bufs=8) as tmp_pool,
            tc.tile_pool(name="osb", bufs=2) as osb_pool,
            tc.tile_pool(name="ps", bufs=8, space="PSUM") as ps_pool,
        ):
            # x tile layout: [128, 32, MT]; dim1 = g*8 + (0..3 xh | 4..7 xl)
            def load_x(m):
                xt = xt_pool.tile([128, 32, MT], f8, tag="xt")
                src = xp.ap()[m * 128:(m + 1) * 128, :]
                nc.sync.dma_start(
                    out=xt[:], in_=src.rearrange("p (c t) -> p c t", c=32))
                return xt

            def load_x_half(xt, m, g, hl):
                # one group's hi (hl=0) or lo (hl=1) chunk quad
                rows = slice(m * 128, (m + 1) * 128)
                c0 = g * 8 + hl * 4
                c1 = c0 + 4
                nc.sync.dma_start(
                    out=xt[:, c0:c1, :],
                    in_=xp.ap()[rows, c0 * MT:c1 * MT].rearrange(
                        "p (c t) -> p c t", c=c1 - c0))

            def load_w13_half(s, g, t):
                # one k-group's rows for one tensor t (0=1h,1=1l,2=3h,3=3l)
                w13, _ = wsb[s]
                c0 = g * 16 + t * 4
                c1 = c0 + 4
                nc.sync.dma_start(
                    out=w13[:, c0:c1, :],
                    in_=wd[s + "13"].ap()[:, c0 * F:c1 * F].rearrange(
                        "p (c f) -> p c f", c=c1 - c0))

            def xslice(xt, kp, lo):
                # kpair kp; lo=False -> xh pair, True -> xl pair
                g, q = kp // 2, kp % 2
                base = g * 8 + (4 if lo else 0) + q * 2
                return xt[:, base:base + 2, :]

            wsb = {}

            def alloc_set(s):
                # w13 dim1 = g*16 + t*4 + chunk, t in (1h, 1l, 3h, 3l)
                w13 = wpool.tile([128, 64, F], f8, tag=f"w13{s}", name="w13")
                # w2 dim1 = (0..3 w2h | 4..7 w2l) f-chunks
                w2 = wpool.tile([128, 8, D], f8, tag=f"w2{s}", name="w2")
                wsb[s] = (w13, w2)

            def load_piece(s, which, fine=False):
                """which 0..3: w13 k-group DMA; 4/5: w2 hi/lo half."""
                w13, w2 = wsb[s]
                if which < 4:
                    g = which
                    nsub = 2 if fine else 1
                    for h in range(nsub):
                        c0 = g * 16 + h * (16 // nsub)
                        c1 = c0 + 16 // nsub
                        nc.sync.dma_start(
                            out=w13[:, c0:c1, :],
                            in_=wd[s + "13"].ap()[:, c0 * F:c1 * F].rearrange(
                                "p (c f) -> p c f", c=c1 - c0))
                else:
                    h = which - 4
                    c0, c1 = h * 4, (h + 1) * 4
                    nc.sync.dma_start(
                        out=w2[:, c0:c1, :],
                        in_=wd[s + "2"].ap()[:, c0 * D:c1 * D].rearrange(
                            "p (c d) -> p c d", c=c1 - c0))

            def wslice(w13, t, kp, fs):
                # tensor t (0=w1h, 1=w1l, 2=w3h, 3=w3l), kpair kp, f block
                g, q = kp // 2, kp % 2
                base = g * 16 + t * 4 + q * 2
                return w13[:, base:base + 2, fs]

            def w2slice(w2, lo, p, jsl):
                base = (4 if lo else 0) + 2 * p
                return w2[:, base:base + 2, jsl]

            def load_set_a_prologue():
                """Set A plus slot-0 x groups interleaved in korder
                consumption order; slot-1 x comes after."""
                alloc_set("a")
                xt0 = xt_pool.tile([128, 32, MT], f8, tag="xt", name="xt0")
                load_x_half(xt0, 0, 0, 0)
                load_x_half(xt0, 0, 0, 1)
                load_piece("a", 0, fine=True)
                nc.sync.dma_start(
                    out=xt0[:, 8:32, :],
                    in_=xp.ap()[0:128, 8 * MT:32 * MT].rearrange(
                        "p (c t) -> p c t", c=24))
                for g in range(1, 4):
                    load_piece("a", g, fine=True)
                load_piece("a", 4)
                load_piece("a", 5)
                xt1 = load_x(1)
                return xt0, xt1

            warm = wpool.tile([128, 64], bf16, tag="warm")
            nc.vector.memset(warm[:], 0.0)
            wps = ps_pool.tile([64, 64], f32, tag="ps", name="wps")
            for _ in range(70):
                nc.tensor.matmul(wps[:], warm[:], warm[:],
                                 start=True, stop=True)

            xt0, xt1 = load_set_a_prologue()
            xts_pre = {0: xt0, 1: xt1}

            msb = wpool.tile([1, 1], i32, tag="meta")
            nc.sync.dma_start(out=msb[:], in_=meta.ap())
            ta_v = nc.snap(nc.values_load(msb[0:1, 0:1]))

            def mm6(w13, xt, ps1, ps3, kp, fs, first, last, term=None):
                # 3-term DoubleRow for both gemms of one (f, kpair):
                # term 0: hi(w)*hi(x), term 1: lo(w)*hi(x), term 2:
                # hi(w)*lo(x). term=None emits all three.
                xhp = xslice(xt, kp, False)
                xlp = xslice(xt, kp, True)
                terms = range(3) if term is None else (term,)
                for t in terms:
                    w1p = wslice(w13, 1 if t == 1 else 0, kp, fs)
                    w3p = wslice(w13, 3 if t == 1 else 2, kp, fs)
                    xpp = xlp if t == 2 else xhp
                    st = first and t == terms[0] if term is None else first
                    sp = last and t == terms[-1] if term is None else last
                    nc.tensor.matmul(ps1[:], w1p, xpp, start=st, stop=sp,
                                     perf_mode=DR)
                    nc.tensor.matmul(ps3[:], w3p, xpp, start=st, stop=sp,
                                     perf_mode=DR)

            def gate(ps1, ps3, hh, hl, f):
                # ACT folds the 2^-7 weight scale back in (silw carries one
                # factor of WS, x3s carries 1/WS^2, so htf = silu(x1)*x3
                # exactly); DVE builds the fp8 hi/lo split of h for stage 2.
                # Sigmoid (not Silu): the bass interpreter only has the
                # former.
                sig = tmp_pool.tile([128, MT], f32, tag="sig")
                nc.scalar.activation(sig[:], ps1[:], ACT.Sigmoid,
                                     scale=1.0 / WS)
                x3s = tmp_pool.tile([128, MT], f32, tag="x3s")
                nc.vector.tensor_scalar_mul(x3s[:], ps3[:], 1.0 / (WS * WS))
                silw = tmp_pool.tile([128, MT], f32, tag="silw")
                nc.vector.tensor_mul(silw[:], ps1[:], sig[:])
                htf = tmp_pool.tile([128, MT], f32, tag="htf")
                nc.vector.tensor_mul(htf[:], silw[:], x3s[:])
                nc.vector.tensor_copy(hh[:, f, :], htf[:])
                nc.vector.tensor_sub(hl[:, f, :], htf[:], hh[:, f, :])

            def stage1(which, xt, hh, hl, korder=False):
                w13, _ = wsb[which]
                if korder:
                    ps1s, ps3s = [], []
                    for f in range(FC):
                        ps1s.append(ps_pool.tile([128, MT], f32, tag="ps",
                                                 name="ps1"))
                        ps3s.append(ps_pool.tile([128, MT], f32, tag="ps",
                                                 name="ps3"))
                    for kp in range(KP):
                        for f in range(FC):
                            fs = slice(f * 128, (f + 1) * 128)
                            mm6(w13, xt, ps1s[f], ps3s[f], kp, fs,
                                kp == 0, kp == KP - 1)
                    for f in range(FC):
                        gate(ps1s[f], ps3s[f], hh, hl, f)
                else:
                    for f in range(FC):
                        fs = slice(f * 128, (f + 1) * 128)
                        ps1 = ps_pool.tile([128, MT], f32, tag="ps")
                        ps3 = ps_pool.tile([128, MT], f32, tag="ps")
                        for kp in range(KP):
                            mm6(w13, xt, ps1, ps3, kp, fs,
                                kp == 0, kp == KP - 1)
                        gate(ps1, ps3, hh, hl, f)

            def s2_mms(w2, hh, hl, pos, ts, p):
                # one f-pair's 12 DoubleRow matmuls for one token block
                tsl = slice(ts * 128, (ts + 1) * 128)
                psl = slice(2 * p, 2 * p + 2)
                hhp = hh[:, psl, tsl]
                hlp = hl[:, psl, tsl]
                for j in range(4):
                    jsl = slice(j * 512, (j + 1) * 512)
                    nc.tensor.matmul(pos[j][:], hhp, w2slice(w2, False, p, jsl),
                                     start=(p == 0), stop=False, perf_mode=DR)
                for j in range(4):
                    jsl = slice(j * 512, (j + 1) * 512)
                    nc.tensor.matmul(pos[j][:], hhp, w2slice(w2, True, p, jsl),
                                     start=False, stop=False, perf_mode=DR)
                for j in range(4):
                    jsl = slice(j * 512, (j + 1) * 512)
                    nc.tensor.matmul(pos[j][:], hlp, w2slice(w2, False, p, jsl),
                                     start=False, stop=(p == FP - 1),
                                     perf_mode=DR)

            def s2_flush(osb, pos, ts, store_m):
                for j in range(2):
                    nc.scalar.activation(
                        osb[:, ts, j * 512:(j + 1) * 512], pos[j][:],
                        ACT.Copy, scale=1.0 / WS)
                for j in range(2, 4):
                    nc.vector.tensor_scalar_mul(
                        osb[:, ts, j * 512:(j + 1) * 512], pos[j][:],
                        1.0 / WS)
                if store_m is not None:
                    rows = slice(store_m * MT + ts * 128,
                                 store_m * MT + (ts + 1) * 128)
                    nc.sync.dma_start(out=out[rows, 0:1024],
                                      in_=osb[:, ts, 0:1024])
                    nc.sync.dma_start(out=out[rows, 1024:2048],
                                      in_=osb[:, ts, 1024:2048])

            def stage2(which, hh, hl, osb, store_m=None):
                # p0 runs for every token block before any p1 so the PE has
                # cover while the last f-chunk's gate chain drains; pos
                # banks are allocated lazily per block to stay inside PSUM.
                _, w2 = wsb[which]
                poss = []
                for ts in range(TS):
                    pos = [ps_pool.tile([128, 512], f32, tag="ps", name="po")
                           for _ in range(4)]
                    poss.append(pos)
                    s2_mms(w2, hh, hl, pos, ts, 0)
                for ts in range(TS):
                    for p in range(1, FP):
                        s2_mms(w2, hh, hl, poss[ts], ts, p)
                    s2_flush(osb, poss[ts], ts, store_m)

            def tile_body(which, xt, osb, m=None, korder=False):
                hh = ht_pool.tile([128, FC, MT], f8, tag="ht")
                hl = ht_pool.tile([128, FC, MT], f8, tag="ht")
                stage1(which, xt, hh, hl, korder=korder)
                stage2(which, hh, hl, osb, store_m=m)

            # B-set pieces spread across the early fixed-A slots so the big
            # weight DMAs never starve the per-slot x loads.
            b_first = min(3, n_a_fixed - 1, nt - 2)
            b_last = max(b_first, min(n_a_fixed, nt - 1) - 2)
            if has_b_slot:
                alloc_set("b")

            def b_load_chunk(m):
                if not has_b_slot:
                    return
                nsl = b_last - b_first + 1
                per = (6 + nsl - 1) // nsl
                i0 = (m - b_first) * per
                for w in range(i0, min(i0 + per, 6)):
                    load_piece("b", w)

            for m in range(nt):
                xt = xts_pre[m] if m in xts_pre else load_x(m)
                osb = osb_pool.tile([128, TS, D], bf16, tag="osb")
                if m < n_a_fixed:
                    tile_body("a", xt, osb, m=m, korder=(m == 0))
                    if b_first <= m <= b_last:
                        b_load_chunk(m)
                elif has_b_slot and m == nt - 1:
                    tile_body("b", xt, osb, m=m)
                else:
                    with tc.If(ta_v > m) as cmp:
                        tile_body("a", xt, osb, m=m)
                    with cmp.Else():
                        tile_body("b", xt, osb, m=m)

    nc.compile()
    return nc


def _get_program(nt: int, naf: int | None = None):
    key = (nt, naf)
    if key not in _cache:
        _cache[key] = _build(nt, naf)
    return _cache[key]


def _assign(counts, nt_cap=None):
    """Greedy: chunk the padded-tile list into per-core runs of <=NT tiles
    spanning <=2 experts. Returns (nt, per-core list of (expert, tile_lo,
    n_tiles) segment pairs) or None if infeasible."""
    E = len(counts)
    pt = [max(1, math.ceil(c / MT)) if c > 0 else 0 for c in counts]
    total = sum(pt)
    nt = math.ceil(total / NCORES)
    for nt_try in (nt, nt + 1):
        segs = [[] for _ in range(NCORES)]
        e, used = 0, 0
        for c in range(NCORES):
            cap = nt_try
            nexp = 0
            while cap > 0 and e < E:
                if pt[e] - used == 0:
                    e += 1
                    used = 0
                    continue
                if nexp == 2:
                    break
                take = min(cap, pt[e] - used)
                segs[c].append((e, used, take))
                used += take
                cap -= take
                nexp += 1
        leftover = total - sum(s[2] for core in segs for s in core)
        if leftover == 0:
            return nt_try, segs
    return None


def _split_f8(a):
    """e4m3 hi/lo split: a ~= hi + lo with ~8-bit mantissa accuracy."""
    hi = a.astype(F8)
    lo = (a - hi.astype(np.float32)).astype(F8)
    return hi, lo


def _pack_w13(w1, w3):
    """[128, 64*F]: row p = cols (g, t, chunk, f), t in (1h,1l,3h,3l)."""
    w1h, w1l = _split_f8(w1 * WS)
    w3h, w3l = _split_f8(w3 * WS)
    # each [D, F] -> [g, chunk, p, F]
    parts = [w.reshape(NG, 4, 128, F) for w in (w1h, w1l, w3h, w3l)]
    arr = np.stack(parts, axis=1)          # [g, t, chunk, p, F]
    return np.ascontiguousarray(
        arr.transpose(3, 0, 1, 2, 4).reshape(128, 64 * F))


def _pack_w2(w2):
    """[128, 8*D]: row p = cols (hl, fchunk, d)."""
    w2h, w2l = _split_f8(w2 * WS)
    arr = np.stack([w2h.reshape(4, 128, D), w2l.reshape(4, 128, D)],
                   axis=0)                 # [hl, c, p, D]
    return np.ascontiguousarray(
        arr.transpose(2, 0, 1, 3).reshape(128, 8 * D))


def kernel(x, num_tokens_per_expert, w1, w2, w3):
    from concourse.bass_utils import run_bass_kernel_spmd

    x = np.asarray(x)
    counts = [int(v) for v in np.asarray(num_tokens_per_expert)]
    w1 = np.asarray(w1)
    w2 = np.asarray(w2)
    w3 = np.asarray(w3)
    T, E = x.shape[0], len(counts)
    starts = np.concatenate([[0], np.cumsum(counts)])[:E].astype(np.int64)

    plan = _assign(counts)
    if plan is None:
        # fallback: expert-parallel (1 segment per core), padded to max tiles
        pt = [max(1, math.ceil(c / MT)) if c > 0 else 0 for c in counts]
        nt = max(pt)
        segs = [[(e, 0, pt[e])] if pt[e] else [] for e in range(min(E, NCORES))]
        segs += [[] for _ in range(NCORES - len(segs))]
        plan = (nt, segs)
    nt, segs = plan
    nt = max(nt, 2)
    # pre-swap so the larger run is A, then size the fixed-A region to the
    # largest remaining B run
    segs = [([s[1], s[0]] if len(s) == 2 and s[1][2] > s[0][2] else list(s))
            for s in segs]
    max_nb = max((s[1][2] for s in segs if len(s) == 2), default=0)
    naf = min(nt - max_nb, nt - 1)
    nc = _get_program(nt, naf)
    PAD_T = nt * MT

    w13p = [_pack_w13(np.ascontiguousarray(w1[e]), np.ascontiguousarray(w3[e]))
            for e in range(E)]
    w2p = [_pack_w2(np.ascontiguousarray(w2[e])) for e in range(E)]

    xT = np.ascontiguousarray(x.T).astype(np.float32)  # [D, T]
    xTh = xT.astype(F8)
    xTl = (xT - xTh.astype(np.float32)).astype(F8)
    # [g, hl, chunk(4), p, T]
    x2g = np.empty((NG, 2, 4, 128, T), dtype=F8)
    x2g[:, 0] = xTh.reshape(NG, 4, 128, T)
    x2g[:, 1] = xTl.reshape(NG, 4, 128, T)
    x2full = x2g.reshape(2 * D, T)

    in_maps = []
    placements = []  # per core: list of (slot, src_lo, n_rows)
    for c in range(NCORES):
        cs = list(segs[c])
        if len(cs) == 2:
            ta = cs[0][2]
            ea, eb = cs[0][0], cs[1][0]
            slot_base = [0, nt - cs[1][2]]
        elif len(cs) == 1:
            ta = nt
            ea = eb = cs[0][0]
            slot_base = [0]
        else:
            ta = nt
            ea = eb = 0
            slot_base = []

        x2c = np.zeros((2 * D, PAD_T), dtype=F8)
        place = []
        for si, (e, tile_lo, ntk) in enumerate(cs):
            src_lo = int(starts[e]) + tile_lo * MT
            src_hi = min(int(starts[e]) + counts[e], src_lo + ntk * MT)
            nrow = src_hi - src_lo
            lo = slot_base[si] * MT
            x2c[:, lo: lo + nrow] = x2full[:, src_lo:src_hi]
            place.append((slot_base[si], src_lo, nrow))
        placements.append(place)
        # repack to xp row-major tile order: rows (m, p), cols (g, hl, c, t)
        xpc = np.ascontiguousarray(
            x2c.reshape(NG, 2, 4, 128, nt, MT)
            .transpose(4, 3, 0, 1, 2, 5)
            .reshape(nt * 128, 32 * MT))
        im = {"xp": xpc, "meta": np.array([[ta]], dtype=np.int32),
              "wa13": w13p[ea], "wa2": w2p[ea],
              "wb13": w13p[eb], "wb2": w2p[eb]}
        in_maps.append(im)

    trace = bool(int(os.environ.get("KERNEL_TRACE", "0")))
    try:
        res = run_bass_kernel_spmd(nc, in_maps, core_ids=list(range(NCORES)),
                                   trace=trace)
    except ModuleNotFoundError:
        res = run_bass_kernel_spmd(nc, in_maps, core_ids=list(range(NCORES)),
                                   trace=False)
    kernel.last_results = res

    out = np.empty((T, D), dtype=np.float32)
    for c in range(NCORES):
        o = np.asarray(res.results[c]["out"])
        for (slot, src_lo, nrow) in placements[c]:
            out[src_lo:src_lo + nrow] = o[slot * MT: slot * MT + nrow].astype(
                np.float32)
    return out


if __name__ == "__main__":
    import simbench
    nc = _build(17, 10)
    simbench.run(nc)
